# revision 29
# baseline (speedup 1.0000x reference)
"""Multi-head self-attention (B=2, L=2048, D=1024, H=16, causal) on 8
Trainium2 NeuronCores.

Sharding: tensor-parallel over heads x data-parallel over batch.
Core c (0..7) handles batch b = c//4 and heads 4*(c%4) .. 4*(c%4)+3.
Each core computes partial = (softmax(qk^T/8) @ v_heads) @ Wo[:, cols]^T of
shape [L, D]; the host sums the 4 partials of each batch group.

Per-core kernel (all matmuls in fp32r = full-rate TF32-like):
  - host supplies x^T so q^T,k^T [256,L] and v [L,256] come straight from
    PE matmuls (no on-device transposes anywhere)
  - scores are computed TRANSPOSED (S^T = k q^T per 128-row key chunk,
    causal tiles only); exp runs on ScalarE directly PSUM->SBUF producing
    P^T in exactly the layout the PV matmul consumes; the 1/sqrt(dh) scale
    and the causal mask of the diagonal block (additive -1e5) are folded in
  - softmax denominators come free as a ones-column appended to v; they are
    broadcast across partitions with a K=1 ones matmul, inverted with a
    fast-reciprocal, and the normalize multiply is fused into the PSUM
    evacuation of the attention output
  - attention output is produced transposed ([64,L] per head), which is
    exactly the lhsT the output projection needs
"""

import numpy as np

B, L, D, H = 2, 2048, 1024, 16
DH = D // H  # 64
HPC = H // 2 // 4  # unused sanity
N_CORES = 8
HEADS_PER_CORE = 4
HD = HEADS_PER_CORE * DH  # 256 head dims per core
NK = D // 128  # 8 contraction chunks
LT = L // 128  # 16 L tiles
NG = L // 512  # 4 column groups

_CACHE = {}


# ---------------------------------------------------------------------------
# walrus compat: this compiler build accepts at most ONE sync-wait command
# per instruction, while TileContext attaches one wait per producer proc.
# Hoist surplus waits onto same-engine NOPs inserted just before the
# offending instruction (identical AND semantics).
# ---------------------------------------------------------------------------
def _split_waits(nc):
    import bass_rust
    import concourse.mybir as mybir

    for fn in nc.m.functions:
        for bb in fn.blocks:
            insts = list(bb.instructions)
            out = []
            changed = False
            for inst in insts:
                si = inst.sync_info
                waits = list(si.on_wait) if si is not None and si.on_wait else []
                if len(waits) > 1:
                    changed = True
                    for w in waits[:-1]:
                        out.append(
                            mybir.InstNoOp(
                                name=nc.get_next_instruction_name(),
                                engine=inst.engine,
                                bass_nofuse=True,
                                sync_info=bass_rust.SyncInfo(
                                    on_wait=[w], on_update=[]
                                ),
                            )
                        )
                    inst.sync_info = bass_rust.SyncInfo(
                        on_wait=[waits[-1]], on_update=list(si.on_update or [])
                    )
                out.append(inst)
            if changed:
                try:
                    bb.instructions = out
                except Exception:
                    bb.instructions.clear()
                    bb.instructions.extend(out)


def _act_reciprocal(nc, mybir, out_ap, in_ap):
    """ScalarE Reciprocal via direct InstActivation construction (the bass
    wrapper refuses it; accuracy here is ~1e-5 rel which is far below the
    fp32r operand rounding of this kernel, and the softmax denominators are
    strictly positive and well-scaled)."""
    AF = mybir.ActivationFunctionType
    eng = nc.scalar
    f32 = mybir.dt.float32
    ins = [
        eng.lower_ap(in_ap),
        eng.lower_ap(nc.const_aps.scalar_like(0.0, in_ap)),
        mybir.ImmediateValue(dtype=f32, value=1.0),
        mybir.ImmediateValue(dtype=f32, value=0.0),
    ]
    return eng.add_instruction(
        mybir.InstActivation(
            name=nc.get_next_instruction_name(),
            func=AF.Reciprocal,
            ins=ins,
            outs=[eng.lower_ap(out_ap)],
        )
    )


def _build_program():
    import concourse.bass as bass
    import concourse.mybir as mybir
    import concourse.tile as tile

    f32 = mybir.dt.float32
    f32r = mybir.dt.float32r
    AF = mybir.ActivationFunctionType

    nc = bass.Bass("TRN2", target_bir_lowering=False, debug=False)
    xT_d = nc.dram_tensor("xT", [D, L], f32, kind="ExternalInput")
    wq_d = nc.dram_tensor("wqT", [D, HD], f32, kind="ExternalInput")
    wk_d = nc.dram_tensor("wkT", [D, HD], f32, kind="ExternalInput")
    wv_d = nc.dram_tensor("wvT", [D, HD], f32, kind="ExternalInput")
    wo_d = nc.dram_tensor("woT", [HD, D], f32, kind="ExternalInput")
    tm_d = nc.dram_tensor("trimask", [128, 128], f32, kind="ExternalInput")
    out_d = nc.dram_tensor("out", [L, D], f32, kind="ExternalOutput")

    with tile.TileContext(nc, pool_alloc_mode="queue") as tc:
        with tc.tile_pool(name="persist", bufs=1) as persist:
            qTr = persist.tile([128, 2, L], f32r)
            kTr = persist.tile([128, 2, L], f32r)
            v_sb = persist.tile([128, LT, HEADS_PER_CORE * (DH + 1)], f32r)
            ones_l = persist.tile([1, 128], f32r)
            tm_t = persist.tile([128, 128], f32)
            woTr = persist.tile([64, HEADS_PER_CORE, D], f32r)

            nc.sync.dma_start(tm_t[:], tm_d[:])

            # ---------------- phase A: projections ----------------
            with (
                tc.tile_pool(name="xtr", bufs=1) as xtrp,
                tc.tile_pool(name="wr", bufs=1) as wrp,
                tc.tile_pool(name="lda", bufs=2) as lda,
                tc.tile_pool(name="psA", bufs=8, space="PSUM") as psA,
            ):
                xTr = [xtrp.tile([128, L], f32r, name=f"xTr{c}") for c in range(NK)]
                wqTr = [wrp.tile([128, HD], f32r, name=f"wqTr{c}") for c in range(NK)]
                wkTr = [wrp.tile([128, HD], f32r, name=f"wkTr{c}") for c in range(NK)]
                wvTr = [wrp.tile([128, HD], f32r, name=f"wvTr{c}") for c in range(NK)]

                for c in range(NK):
                    sw = lda.tile([128, HD], f32, tag="wstage")
                    nc.sync.dma_start(sw[:], wq_d[c * 128 : (c + 1) * 128, :])
                    nc.vector.tensor_copy(wqTr[c][:], sw[:])
                    st = lda.tile([128, L], f32, tag="xstage")
                    nc.sync.dma_start(st[:], xT_d[c * 128 : (c + 1) * 128, :])
                    nc.scalar.copy(xTr[c][:], st[:])
                    sw = lda.tile([128, HD], f32, tag="wstage")
                    nc.sync.dma_start(sw[:], wk_d[c * 128 : (c + 1) * 128, :])
                    nc.vector.tensor_copy(wkTr[c][:], sw[:])
                    sw = lda.tile([128, HD], f32, tag="wstage")
                    nc.sync.dma_start(sw[:], wv_d[c * 128 : (c + 1) * 128, :])
                    nc.vector.tensor_copy(wvTr[c][:], sw[:])
                for h in range(HEADS_PER_CORE):
                    sw2 = lda.tile([64, D], f32, tag="wostage")
                    nc.sync.dma_start(sw2[:], wo_d[h * 64 : (h + 1) * 64, :])
                    nc.vector.tensor_copy(woTr[:, h, :], sw2[:])
                onesf = lda.tile([1, 128], f32, tag="onesf")
                nc.vector.memset(onesf[:], 1.0)
                nc.vector.tensor_copy(ones_l[:], onesf[:])

                # qT, kT: [256, L] as head-pair chunks [128, 2, L]
                for j in range(2):
                    for wt, dst in ((wqTr, qTr), (wkTr, kTr)):
                        for g in range(NG):
                            ps = psA.tile([128, 512], f32, tag="psqk")
                            for c in range(NK):
                                nc.tensor.matmul(
                                    ps[:],
                                    wt[c][:, j * 128 : (j + 1) * 128],
                                    xTr[c][:, g * 512 : (g + 1) * 512],
                                    start=(c == 0),
                                    stop=(c == NK - 1),
                                )
                            nc.vector.tensor_copy(dst[:, j, g * 512 : (g + 1) * 512], ps[:])

                # v: [L, 256] with a ones column per head ([.., 65h+64])
                onesv = lda.tile([128, HEADS_PER_CORE], f32, tag="onesv")
                nc.vector.memset(onesv[:], 1.0)
                for t in range(LT):
                    ps = psA.tile([128, 512], f32, tag="psqk")
                    for c in range(NK):
                        nc.tensor.matmul(
                            ps[:, 0:HD],
                            xTr[c][:, t * 128 : (t + 1) * 128],
                            wvTr[c][:],
                            start=(c == 0),
                            stop=(c == NK - 1),
                        )
                    vdst = v_sb[:, t, :].rearrange(
                        "p (h u) -> p h u", u=DH + 1
                    )
                    nc.vector.tensor_copy(
                        vdst[:, :, 0:DH],
                        ps[:, 0:HD].rearrange("p (h u) -> p h u", u=DH),
                    )
                    nc.vector.tensor_copy(
                        vdst[:, :, DH : DH + 1],
                        onesv[:].rearrange("p (h u) -> p h u", u=1),
                    )

            # ---------------- phase B: attention per head ----------------
            with (
                tc.tile_pool(name="otp", bufs=1) as otp,
                tc.tile_pool(name="ptp", bufs=3) as ptp,
                tc.tile_pool(name="rsp", bufs=2) as rsp,
                tc.tile_pool(name="bcp", bufs=4) as bcp,
                tc.tile_pool(name="psST", bufs=2, space="PSUM") as psST,
                tc.tile_pool(name="psPV", bufs=1, space="PSUM") as psPV,
            ):
                ot_sb = otp.tile([64, HEADS_PER_CORE, L], f32r)
                for h in range(HEADS_PER_CORE):
                    hp, ho = h // 2, 64 * (h % 2)
                    pvs = [
                        psPV.tile([65, 512], f32, name=f"pv_h{h}_{g}", tag=f"pv{g}")
                        for g in range(NG)
                    ]
                    for m in range(LT):
                        c0 = 128 * m
                        w = L - c0
                        PT = ptp.tile([128, L], f32r, tag="pt")
                        nsub = (w + 1023) // 1024
                        for sub in range(nsub):
                            s0 = c0 + 1024 * sub
                            sw = min(1024, L - s0)
                            stp = psST.tile([128, 1024], f32, tag="st")
                            for nn in range((sw + 511) // 512):
                                n0 = s0 + 512 * nn
                                nw = min(512, s0 + sw - n0)
                                nc.tensor.matmul(
                                    stp[:, nn * 512 : nn * 512 + nw],
                                    kTr[ho : ho + 64, hp, c0 : c0 + 128],
                                    qTr[ho : ho + 64, hp, n0 : n0 + nw],
                                    start=True,
                                    stop=True,
                                )
                            if sub == 0:
                                nc.vector.tensor_add(
                                    stp[:, 0:128], stp[:, 0:128], tm_t[:]
                                )
                            nc.scalar.activation(
                                PT[:, s0 - c0 : s0 - c0 + sw],
                                stp[:, 0:sw],
                                AF.Exp,
                                scale=0.125,
                            )
                        for g in range(NG):
                            gs = 512 * g
                            if gs + 512 <= c0:
                                continue
                            r0 = max(gs, c0)
                            last = m == min(LT - 1, 4 * g + 3)
                            nc.tensor.matmul(
                                pvs[g][:, r0 - gs : 512],
                                v_sb[:, m, h * 65 : h * 65 + 65],
                                PT[:, r0 - c0 : gs + 512 - c0],
                                start=(m == 0),
                                stop=last,
                            )
                            if last:
                                # g-block complete: evacuate + normalize now so
                                # the chain overlaps the remaining m iterations
                                rs_row = rsp.tile([1, 512], f32r, tag="rs")
                                nc.vector.tensor_copy(rs_row[:], pvs[g][64:65, :])
                                bc_ps = psST.tile([128, 512], f32, tag="st")
                                nc.tensor.matmul(
                                    bc_ps[:], ones_l[:], rs_row[:],
                                    start=True, stop=True,
                                )
                                bc_sb = bcp.tile([128, 512], f32, tag="bc")
                                _act_reciprocal(nc, mybir, bc_sb[:], bc_ps[:])
                                nc.vector.tensor_mul(
                                    ot_sb[:, h, 512 * g : 512 * g + 512],
                                    pvs[g][0:64, :],
                                    bc_sb[0:64, :],
                                )

            # ---------------- phase C: output projection ----------------
            with (
                tc.tile_pool(name="outst", bufs=3) as outst,
                tc.tile_pool(name="psC", bufs=4, space="PSUM") as psC,
            ):
                for t in range(LT):
                    stage = outst.tile([128, D], f32, tag="ostage")
                    for n2 in range(2):
                        ps = psC.tile([128, 512], f32, tag="psc")
                        for h in range(HEADS_PER_CORE):
                            nc.tensor.matmul(
                                ps[:],
                                ot_sb[:, h, t * 128 : (t + 1) * 128],
                                woTr[:, h, n2 * 512 : (n2 + 1) * 512],
                                start=(h == 0),
                                stop=(h == HEADS_PER_CORE - 1),
                            )
                        nc.scalar.copy(
                            stage[:, n2 * 512 : (n2 + 1) * 512], ps[:]
                        )
                    nc.sync.dma_start(
                        out_d[t * 128 : (t + 1) * 128, :], stage[:]
                    )



    _split_waits(nc)
    return nc



def _build_runner(nc):
    """Build the sharded PJRT executable once (mirrors
    bass2jax.run_bass_via_pjrt) and return a callable in_maps -> results."""
    import jax
    import numpy as _np
    from jax.sharding import Mesh, PartitionSpec
    from jax.experimental.shard_map import shard_map
    from concourse import bass2jax, mybir

    bass2jax.install_neuronx_cc_hook()
    partition_name = (
        nc.partition_id_tensor.name if nc.partition_id_tensor else None
    )
    in_names, out_names, out_avals, zero_outs = [], [], [], []
    for alloc in nc.m.functions[0].allocations:
        if not isinstance(alloc, mybir.MemoryLocationSet):
            continue
        name = alloc.memorylocations[0].name
        if alloc.kind == "ExternalInput":
            if name != partition_name:
                in_names.append(name)
        elif alloc.kind == "ExternalOutput":
            out_names.append(name)
            shape = tuple(alloc.tensor_shape)
            dtype = mybir.dt.np(alloc.dtype)
            out_avals.append(jax.core.ShapedArray(shape, dtype))
            zero_outs.append(_np.zeros(shape, dtype))
    n_params = len(in_names)
    n_outs = len(out_names)
    all_in_names = list(in_names) + list(out_names)
    if partition_name is not None:
        all_in_names.append(partition_name)
    donate = tuple(range(n_params, n_params + n_outs))

    def _body(*args):
        operands = list(args)
        if partition_name is not None:
            operands.append(bass2jax.partition_id_tensor())
        outs = bass2jax._bass_exec_p.bind(
            *operands,
            out_avals=tuple(out_avals),
            in_names=tuple(all_in_names),
            out_names=tuple(out_names),
            lowering_input_output_aliases=(),
            sim_require_finite=True,
            sim_require_nnan=True,
            nc=nc,
        )
        return tuple(outs)

    devices = jax.devices()[:N_CORES]
    mesh = Mesh(_np.asarray(devices), ("core",))
    in_specs = (PartitionSpec("core"),) * (n_params + n_outs)
    out_specs = (PartitionSpec("core"),) * n_outs
    sharded = jax.jit(
        shard_map(
            _body, mesh=mesh, in_specs=in_specs, out_specs=out_specs,
            check_rep=False,
        ),
        donate_argnums=donate,
        keep_unused=True,
    )

    def run(in_maps):
        concat_in = [
            _np.concatenate([_np.asarray(m[nm]) for m in in_maps], axis=0)
            for nm in in_names
        ]
        concat_zeros = [
            _np.zeros((N_CORES * z.shape[0], *z.shape[1:]), z.dtype)
            for z in zero_outs
        ]
        out_arrs = sharded(*concat_in, *concat_zeros)
        return [
            {
                nm: _np.asarray(out_arrs[i]).reshape(
                    N_CORES, *out_avals[i].shape
                )[c]
                for i, nm in enumerate(out_names)
            }
            for c in range(N_CORES)
        ]

    return run


def _numpy_ref(x, attn_mask, Wq, Wk, Wv, Wo):
    xb, Lb, Db = x.shape
    dh = Db // H
    x64 = x.astype(np.float64)
    q = (x64 @ Wq.T.astype(np.float64)).reshape(xb, Lb, H, dh)
    k = (x64 @ Wk.T.astype(np.float64)).reshape(xb, Lb, H, dh)
    v = (x64 @ Wv.T.astype(np.float64)).reshape(xb, Lb, H, dh)
    scores = np.einsum("blhd,bmhd->bhlm", q, k) / np.sqrt(dh)
    scores = np.where(attn_mask[None, None, :, :] == 0, -np.inf, scores)
    scores -= scores.max(axis=-1, keepdims=True)
    e = np.exp(scores)
    attn = e / e.sum(axis=-1, keepdims=True)
    out = np.einsum("bhlm,bmhd->blhd", attn, v).reshape(xb, Lb, Db)
    return (out @ Wo.T.astype(np.float64)).astype(x.dtype)


def _trimask():
    j = np.arange(128)
    return np.where(j[None, :] >= j[:, None], 0.0, -1.0e5).astype(np.float32)


def _make_in_maps(x, Wq, Wk, Wv, Wo):
    tm = _trimask()
    xT = [np.ascontiguousarray(x[b].T).astype(np.float32, copy=False) for b in range(B)]
    WqT = np.ascontiguousarray(Wq.T).astype(np.float32, copy=False)
    WkT = np.ascontiguousarray(Wk.T).astype(np.float32, copy=False)
    WvT = np.ascontiguousarray(Wv.T).astype(np.float32, copy=False)
    in_maps = []
    for c in range(N_CORES):
        b = c // 4
        s0 = HD * (c % 4)
        sel = slice(s0, s0 + HD)
        in_maps.append(
            {
                "xT": xT[b],
                "wqT": WqT[:, sel],
                "wkT": WkT[:, sel],
                "wvT": WvT[:, sel],
                "woT": np.ascontiguousarray(Wo[:, sel].T).astype(np.float32, copy=False),
                "trimask": tm,
            }
        )
    return in_maps


def kernel(x, attn_mask, Wq, Wk, Wv, Wo):
    x = np.asarray(x)
    attn_mask = np.asarray(attn_mask)
    Wq, Wk, Wv, Wo = (np.asarray(a) for a in (Wq, Wk, Wv, Wo))
    causal = x.shape == (B, L, D) and np.array_equal(
        attn_mask != 0, np.tril(np.ones((L, L), dtype=bool))
    )
    if not causal:
        return _numpy_ref(x, attn_mask, Wq, Wk, Wv, Wo)

    if "run" not in _CACHE:
        _CACHE["run"] = _build_runner(_build_program())
    in_maps = _make_in_maps(x, Wq, Wk, Wv, Wo)
    results = _CACHE["run"](in_maps)
    out = np.zeros((B, L, D), dtype=np.float32)
    for c in range(N_CORES):
        out[c // 4] += results[c]["out"]
    return out
